# revision 11
# baseline (speedup 1.0000x reference)
"""Category-routed 2-layer MLP (MoE-style routing) on 8 Trainium2 cores.

Problem: out[i] = relu(x[i] @ W1[c] + b1[c]) @ W2[c] + b2[c], c = cat_ids[i],
with B=131072 tokens, C=16 categories, D_IN=256, D_H=1024, D_OUT=256.

Sharding (category-parallel, chosen over the data-parallel hint because it
cuts per-core weight traffic 8x): each core owns 2 whole categories; tokens
are gathered to their category's core on the host and handed to the device
transposed ([D_IN, T]) so the contraction dim sits on SBUF partitions.
Per-core segments are padded to a uniform compile-time capacity so a single
SPMD program serves all 8 cores.

Per 512-token chunk (all matmuls float32r — full PE rate, absmax err ~1e-4;
fp8 was measured failing the 2e-2 gate and bf16 measured *slower* than f32r
on this silicon):
  L1: psum[h_tile, t] += W1_tile.T @ xT_tile        (16 MMs, N=512)
  relu+b1: psum -> SBUF hT, split between ScalarE and VectorE (relu_acts=5)
      - a single engine would bottleneck: ACT inst = (N+352)/1.2 ns, so
        8 tiles on ACT alone is ~5.8 us/chunk, at/above the PE's ~5 us.
  L2 (W2-stationary): psum[o_tile, t] += W2_tile.T @ hT_tile (16 MMs, N=512)
      - vs the hT-stationary form (32 N=256 MMs): stationary W2 tiles are
        SBUF-resident and their weight loads hide under 512-cycle MMs,
        instead of loading a fresh hT tile into the PE every 256-cycle MM.
  b2 add: per-partition bias (o on partitions), split ACT/DVE; output is
  written o-major ([256, T] per core) and un-transposed on the host during
  unshard (symmetric with the host-side x transpose on the way in).
  The output DMA is issued from the Activation engine's hardware DGE queue
  while the x-input DMA rides the sync engine's queue — splitting the
  ~1 MB/chunk of traffic across two hardware queues measured a consistent
  ~5% win over a single queue (6/6 interleaved A/B rounds).
Layer 2 runs one chunk behind layer 1 (software pipeline) so the PE never
waits on a chunk's relu chain.

Measured on the 8-core axon mesh: 165448 ns vs 192374/203803 ns for the
hT-stationary all-ACT baseline (same-session comparisons; absolute numbers
drift up to ~35% with machine thermal/power state).
"""

import numpy as np
from contextlib import ExitStack

import concourse.bacc as bacc
import concourse.tile as tile
from concourse import mybir
from concourse.bass_utils import run_bass_kernel_spmd

N_CORES = 8
P = 128
CHUNK = 512
F32 = mybir.dt.float32
F32R = mybir.dt.float32r
RELU = mybir.ActivationFunctionType.Relu
IDENT = mybir.ActivationFunctionType.Identity


def build_program(seg_caps, d_in, d_h, d_out, repeat=1, relu_acts=5,
                  bias_acts=1, ps1_bufs=4, ps2_bufs=4, xp_bufs=4, op_bufs=4,
                  hp_bufs=2, interleave=False, w_bf16=False,
                  out_dma_act=True):
    n_seg = len(seg_caps)
    T = sum(seg_caps)
    KD = d_in // P    # contraction tiles for layer 1 (2)
    MH = d_h // P     # h tiles (8)
    OT = d_out // P   # output o tiles (2)

    nc = bacc.Bacc("TRN2", target_bir_lowering=False, debug=False,
                   num_devices=N_CORES)
    xT = nc.declare_dram_parameter("xT", [d_in, T], F32R, isOutput=False)
    W1 = nc.declare_dram_parameter("W1", [n_seg, d_in, d_h], F32R, isOutput=False)
    b1 = nc.declare_dram_parameter("b1", [n_seg, d_h], F32, isOutput=False)
    W2 = nc.declare_dram_parameter("W2", [n_seg, d_h, d_out], F32R, isOutput=False)
    b2T = nc.declare_dram_parameter("b2T", [n_seg, P, OT], F32, isOutput=False)
    out = nc.declare_dram_parameter("outT", [d_out, T], F32, isOutput=True)

    xT_v = xT.ap().rearrange("(kd p) t -> p kd t", p=P)        # [P, KD, T]
    w1_v = W1.ap().rearrange("s (kd p) h -> s p kd h", p=P)    # [S, P, KD, d_h]
    w2_v = W2.ap().rearrange("s (kh p) o -> s p kh o", p=P)    # [S, P, MH, d_out]
    b1_v = b1.ap().rearrange("s (mh p) -> s p mh", p=P)        # [S, P, MH]
    b2_v = b2T.ap()                                            # [S, P, OT]
    out_v = out.ap().rearrange("(g p) t -> p g t", p=P)        # [P, OT, T]

    chunk_list = []
    for s in range(n_seg):
        off_t, rem = sum(seg_caps[:s]), seg_caps[s]
        while rem > 0:
            sz = min(CHUNK, rem)
            chunk_list.append((s, off_t, sz))
            off_t += sz
            rem -= sz

    with tile.TileContext(nc) as tc, ExitStack() as ctx:
        const = ctx.enter_context(tc.tile_pool(name="const", bufs=1))
        xpool = ctx.enter_context(tc.tile_pool(name="xp", bufs=xp_bufs))
        hpool = ctx.enter_context(tc.tile_pool(name="hp", bufs=hp_bufs))
        opool = ctx.enter_context(tc.tile_pool(name="op", bufs=op_bufs))
        ps1 = ctx.enter_context(tc.tile_pool(name="ps1", bufs=ps1_bufs, space="PSUM"))
        ps2 = ctx.enter_context(tc.tile_pool(name="ps2", bufs=ps2_bufs, space="PSUM"))

        if w_bf16:
            wstage = ctx.enter_context(tc.tile_pool(name="wstage", bufs=2))
        WDT = mybir.dt.bfloat16 if w_bf16 else F32R

        w1_sb, w2_sb, b1_sb, b2_sb = [], [], [], []
        for s in range(n_seg):
            w1_t = const.tile([P, KD, d_h], WDT, tag=f"w1_{s}")
            if w_bf16:
                st1 = wstage.tile([P, KD, d_h], F32R, tag="wst1")
                nc.sync.dma_start(out=st1[:], in_=w1_v[s])
                nc.vector.tensor_copy(w1_t[:], st1[:])
            else:
                nc.sync.dma_start(out=w1_t[:], in_=w1_v[s])
            w1_sb.append(w1_t)
            w2_t = const.tile([P, MH, d_out], WDT, tag=f"w2_{s}")
            if w_bf16:
                st2 = wstage.tile([P, MH, d_out], F32R, tag="wst2")
                nc.sync.dma_start(out=st2[:], in_=w2_v[s])
                nc.vector.tensor_copy(w2_t[:], st2[:])
            else:
                nc.sync.dma_start(out=w2_t[:], in_=w2_v[s])
            w2_sb.append(w2_t)
            b1_t = const.tile([P, MH], F32, tag=f"b1_{s}")
            nc.sync.dma_start(out=b1_t[:], in_=b1_v[s])
            b1_sb.append(b1_t)
            b2_t = const.tile([P, OT], F32, tag=f"b2_{s}")
            nc.sync.dma_start(out=b2_t[:], in_=b2_v[s])
            b2_sb.append(b2_t)

        def emit_bias2(ot_sb, pt2, s, ot):
            if ot < bias_acts:
                nc.scalar.activation(
                    ot_sb[:, ot, :], pt2[:], IDENT,
                    bias=b2_sb[s][:, ot:ot + 1])
            else:
                nc.vector.tensor_scalar(
                    ot_sb[:, ot, :], pt2[:],
                    b2_sb[s][:, ot:ot + 1], None,
                    mybir.AluOpType.add)

        def emit_l2(hT, s, tok0, sz):
            # layer 2: outT[o, t] = W2.T @ hT + b2, o-major
            ot_sb = opool.tile([P, OT, sz], F32, tag="ot")
            if interleave:
                # two independent accumulation chains alternating PSUM banks
                pt2s = [ps2.tile([P, sz], F32, tag="ps2", name=f"ps2_{ot}")
                        for ot in range(OT)]
                for kh in range(MH):
                    for ot in range(OT):
                        nc.tensor.matmul(
                            pt2s[ot][:],
                            lhsT=w2_sb[s][:, kh, ot * P:(ot + 1) * P],
                            rhs=hT[:, kh, :sz],
                            start=(kh == 0), stop=(kh == MH - 1))
                for ot in range(OT):
                    emit_bias2(ot_sb, pt2s[ot], s, ot)
            else:
                for ot in range(OT):
                    pt2 = ps2.tile([P, sz], F32, tag="ps2")
                    for kh in range(MH):
                        nc.tensor.matmul(
                            pt2[:],
                            lhsT=w2_sb[s][:, kh, ot * P:(ot + 1) * P],
                            rhs=hT[:, kh, :sz],
                            start=(kh == 0), stop=(kh == MH - 1))
                    emit_bias2(ot_sb, pt2, s, ot)
            # out-DMA optionally on the Activation engine's HWDGE queue so
            # input and output transfers ride two hardware queues
            dma_eng = nc.scalar if out_dma_act else nc.sync
            dma_eng.dma_start(out=out_v[:, :, tok0:tok0 + sz], in_=ot_sb[:])

        for _rep in range(repeat):
            pending = None
            for (s, tok0, sz) in chunk_list:
                xt = xpool.tile([P, KD, sz], F32R, tag="xt")
                nc.sync.dma_start(out=xt[:], in_=xT_v[:, :, tok0:tok0 + sz])

                def emit_relu(hT, pt, s, mh):
                    if mh < relu_acts:
                        nc.scalar.activation(
                            hT[:, mh, :], pt[:], RELU,
                            bias=b1_sb[s][:, mh:mh + 1])
                    else:
                        nc.vector.tensor_scalar(
                            hT[:, mh, :], pt[:],
                            b1_sb[s][:, mh:mh + 1], 0.0,
                            mybir.AluOpType.add, mybir.AluOpType.max)

                hT = hpool.tile([P, MH, sz], F32R, tag="hT")
                if interleave:
                    # pairs of h tiles: two accumulation chains in flight
                    for m in range(MH // 2):
                        pts = [ps1.tile([P, sz], F32, tag="ps1",
                                        name=f"ps1_{j}") for j in range(2)]
                        for kd in range(KD):
                            for j in range(2):
                                nc.tensor.matmul(
                                    pts[j][:],
                                    lhsT=w1_sb[s][:, kd,
                                                  (2 * m + j) * P:
                                                  (2 * m + j + 1) * P],
                                    rhs=xt[:, kd, :],
                                    start=(kd == 0), stop=(kd == KD - 1))
                        for j in range(2):
                            emit_relu(hT, pts[j], s, 2 * m + j)
                else:
                    for mh in range(MH):
                        pt = ps1.tile([P, sz], F32, tag="ps1")
                        for kd in range(KD):
                            nc.tensor.matmul(
                                pt[:],
                                lhsT=w1_sb[s][:, kd, mh * P:(mh + 1) * P],
                                rhs=xt[:, kd, :],
                                start=(kd == 0), stop=(kd == KD - 1))
                        emit_relu(hT, pt, s, mh)

                if pending is not None:
                    emit_l2(*pending)
                pending = (hT, s, tok0, sz)
            emit_l2(*pending)

    nc.compile()
    return nc


def _route(cat_ids, n_cat):
    counts = np.bincount(cat_ids, minlength=n_cat)
    order = np.argsort(counts, kind="stable")[::-1]
    seg_cats = [order[:N_CORES], order[n_cat - 1:N_CORES - 1:-1]]
    caps = []
    for j in range(2):
        mx = int(counts[seg_cats[j]].max())
        caps.append(max(CHUNK, -(-mx // CHUNK) * CHUNK))
    return seg_cats, caps, counts


_PROG_CACHE = {}


def make_in_maps(x, cat_ids, W1, b1, W2, b2):
    x = np.ascontiguousarray(np.asarray(x, dtype=np.float32))
    cat_ids = np.asarray(cat_ids)
    W1 = np.asarray(W1, dtype=np.float32)
    b1 = np.asarray(b1, dtype=np.float32)
    W2 = np.asarray(W2, dtype=np.float32)
    b2 = np.asarray(b2, dtype=np.float32)

    d_in = x.shape[1]
    n_cat, _, d_h = W1.shape
    d_out = W2.shape[2]

    seg_cats, caps, _counts = _route(cat_ids, n_cat)
    T = sum(caps)

    idx_per_core = []
    in_maps = []
    for i in range(N_CORES):
        cats = [int(seg_cats[0][i]), int(seg_cats[1][i])]
        idxs = [np.flatnonzero(cat_ids == c) for c in cats]
        idx_per_core.append(idxs)
        xT_i = np.zeros((d_in, T), dtype=np.float32)
        off = 0
        for j, (c, idx) in enumerate(zip(cats, idxs)):
            xT_i[:, off:off + len(idx)] = x[idx].T
            off += caps[j]
        # b2T[s][p, ot] = b2[cats[s]][ot*P + p]  (o-major per-partition bias)
        b2T = np.ascontiguousarray(
            b2[cats].reshape(2, d_out // P, P).transpose(0, 2, 1))
        in_maps.append({
            "xT": xT_i,
            "W1": np.ascontiguousarray(W1[cats]),
            "b1": np.ascontiguousarray(b1[cats]),
            "W2": np.ascontiguousarray(W2[cats]),
            "b2T": b2T,
        })
    return in_maps, idx_per_core, caps, (d_in, d_h, d_out)


def unshard_out(results, idx_per_core, caps, B, d_out):
    out_full = np.empty((B, d_out), dtype=np.float32)
    for i in range(N_CORES):
        o = results[i]["outT"]  # [d_out, T]
        off = 0
        for j, idx in enumerate(idx_per_core[i]):
            out_full[idx] = o[:, off:off + len(idx)].T
            off += caps[j]
    return out_full


def kernel(x, cat_ids, W1, b1, W2, b2):
    in_maps, idx_per_core, caps, (d_in, d_h, d_out) = make_in_maps(
        x, cat_ids, W1, b1, W2, b2)

    key = (tuple(caps), d_in, d_h, d_out)
    if key not in _PROG_CACHE:
        _PROG_CACHE[key] = build_program(caps, d_in, d_h, d_out)
    nc = _PROG_CACHE[key]

    res = run_bass_kernel_spmd(nc, in_maps, list(range(N_CORES)))
    return unshard_out(res.results, idx_per_core, caps,
                       np.asarray(x).shape[0], d_out)


# revision 18
# speedup vs baseline: 1.1661x; 1.1661x over previous
"""Category-routed 2-layer MLP (MoE-style routing) on 8 Trainium2 cores.

Problem: out[i] = relu(x[i] @ W1[c] + b1[c]) @ W2[c] + b2[c], c = cat_ids[i],
with B=131072 tokens, C=16 categories, D_IN=256, D_H=1024, D_OUT=256.

Sharding (category-parallel, chosen over the data-parallel hint because it
cuts per-core weight traffic 8x): each core owns 2 whole categories; tokens
are gathered to their category's core on the host and handed to the device
transposed ([D_IN, T]) so the contraction dim sits on SBUF partitions.
Per-core segments are padded to a uniform compile-time capacity so a single
SPMD program serves all 8 cores.

Per 512-token chunk (all matmuls float32r — full PE rate, absmax err ~1e-4;
fp8 was measured failing the 2e-2 gate and bf16 measured *slower* than f32r
on this silicon):
  L1: psum[h_tile, t] += W1_tile.T @ xT_tile        (16 MMs, N=512)
  relu+b1: psum -> SBUF hT, split between ScalarE and VectorE (relu_acts=5)
      - a single engine would bottleneck: ACT inst = (N+352)/1.2 ns, so
        8 tiles on ACT alone is ~5.8 us/chunk, at/above the PE's ~5 us.
  L2 (W2-stationary): psum[o_tile, t] += W2_tile.T @ hT_tile (16 MMs, N=512)
      - vs the hT-stationary form (32 N=256 MMs): stationary W2 tiles are
        SBUF-resident and their weight loads hide under 512-cycle MMs,
        instead of loading a fresh hT tile into the PE every 256-cycle MM.
  b2 add: per-partition bias (o on partitions), split ACT/DVE; output is
  written o-major ([256, T] per core) and un-transposed on the host during
  unshard (symmetric with the host-side x transpose on the way in).
  The output DMA is issued from the Activation engine's hardware DGE queue
  while the x-input DMA rides the sync engine's queue — splitting the
  ~1 MB/chunk of traffic across two hardware queues measured a consistent
  ~5% win over a single queue (6/6 interleaved A/B rounds).
Layer 2 runs two chunks behind layer 1 (software pipeline, l2_lag=2 with
hp_bufs=4) so the PE never waits on a chunk's relu chain — the extra chunk
of slack measured ~4% over a lag of one in order-rotated interleaved A/B.

Measured on the 8-core axon mesh: 165448 ns vs 192374/203803 ns for the
hT-stationary all-ACT baseline (same-session comparisons; absolute numbers
drift up to ~35% with machine thermal/power state).
"""

import numpy as np
from contextlib import ExitStack

import concourse.bacc as bacc
import concourse.tile as tile
from concourse import mybir
from concourse.bass_utils import run_bass_kernel_spmd

N_CORES = 8
P = 128
CHUNK = 512
F32 = mybir.dt.float32
F32R = mybir.dt.float32r
RELU = mybir.ActivationFunctionType.Relu
IDENT = mybir.ActivationFunctionType.Identity


def build_program(seg_caps, d_in, d_h, d_out, repeat=1, relu_acts=5,
                  bias_acts=1, ps1_bufs=4, ps2_bufs=4, xp_bufs=4, op_bufs=4,
                  hp_bufs=4, interleave=False, w_bf16=False,
                  out_dma_act=True, l2_lag=2, split_out_dma=False):
    n_seg = len(seg_caps)
    T = sum(seg_caps)
    KD = d_in // P    # contraction tiles for layer 1 (2)
    MH = d_h // P     # h tiles (8)
    OT = d_out // P   # output o tiles (2)

    nc = bacc.Bacc("TRN2", target_bir_lowering=False, debug=False,
                   num_devices=N_CORES)
    xT = nc.declare_dram_parameter("xT", [d_in, T], F32R, isOutput=False)
    W1 = nc.declare_dram_parameter("W1", [n_seg, d_in, d_h], F32R, isOutput=False)
    b1 = nc.declare_dram_parameter("b1", [n_seg, d_h], F32, isOutput=False)
    W2 = nc.declare_dram_parameter("W2", [n_seg, d_h, d_out], F32R, isOutput=False)
    b2T = nc.declare_dram_parameter("b2T", [n_seg, P, OT], F32, isOutput=False)
    out = nc.declare_dram_parameter("outT", [d_out, T], F32, isOutput=True)

    xT_v = xT.ap().rearrange("(kd p) t -> p kd t", p=P)        # [P, KD, T]
    w1_v = W1.ap().rearrange("s (kd p) h -> s p kd h", p=P)    # [S, P, KD, d_h]
    w2_v = W2.ap().rearrange("s (kh p) o -> s p kh o", p=P)    # [S, P, MH, d_out]
    b1_v = b1.ap().rearrange("s (mh p) -> s p mh", p=P)        # [S, P, MH]
    b2_v = b2T.ap()                                            # [S, P, OT]
    out_v = out.ap().rearrange("(g p) t -> p g t", p=P)        # [P, OT, T]

    chunk_list = []
    for s in range(n_seg):
        off_t, rem = sum(seg_caps[:s]), seg_caps[s]
        while rem > 0:
            sz = min(CHUNK, rem)
            chunk_list.append((s, off_t, sz))
            off_t += sz
            rem -= sz

    with tile.TileContext(nc) as tc, ExitStack() as ctx:
        const = ctx.enter_context(tc.tile_pool(name="const", bufs=1))
        xpool = ctx.enter_context(tc.tile_pool(name="xp", bufs=xp_bufs))
        hpool = ctx.enter_context(tc.tile_pool(name="hp", bufs=hp_bufs))
        opool = ctx.enter_context(tc.tile_pool(name="op", bufs=op_bufs))
        ps1 = ctx.enter_context(tc.tile_pool(name="ps1", bufs=ps1_bufs, space="PSUM"))
        ps2 = ctx.enter_context(tc.tile_pool(name="ps2", bufs=ps2_bufs, space="PSUM"))

        if w_bf16:
            wstage = ctx.enter_context(tc.tile_pool(name="wstage", bufs=2))
        WDT = mybir.dt.bfloat16 if w_bf16 else F32R

        w1_sb, w2_sb, b1_sb, b2_sb = [], [], [], []
        for s in range(n_seg):
            w1_t = const.tile([P, KD, d_h], WDT, tag=f"w1_{s}")
            if w_bf16:
                st1 = wstage.tile([P, KD, d_h], F32R, tag="wst1")
                nc.sync.dma_start(out=st1[:], in_=w1_v[s])
                nc.vector.tensor_copy(w1_t[:], st1[:])
            else:
                nc.sync.dma_start(out=w1_t[:], in_=w1_v[s])
            w1_sb.append(w1_t)
            w2_t = const.tile([P, MH, d_out], WDT, tag=f"w2_{s}")
            if w_bf16:
                st2 = wstage.tile([P, MH, d_out], F32R, tag="wst2")
                nc.sync.dma_start(out=st2[:], in_=w2_v[s])
                nc.vector.tensor_copy(w2_t[:], st2[:])
            else:
                nc.sync.dma_start(out=w2_t[:], in_=w2_v[s])
            w2_sb.append(w2_t)
            b1_t = const.tile([P, MH], F32, tag=f"b1_{s}")
            nc.sync.dma_start(out=b1_t[:], in_=b1_v[s])
            b1_sb.append(b1_t)
            b2_t = const.tile([P, OT], F32, tag=f"b2_{s}")
            nc.sync.dma_start(out=b2_t[:], in_=b2_v[s])
            b2_sb.append(b2_t)

        def emit_bias2(ot_sb, pt2, s, ot):
            if ot < bias_acts:
                nc.scalar.activation(
                    ot_sb[:, ot, :], pt2[:], IDENT,
                    bias=b2_sb[s][:, ot:ot + 1])
            else:
                nc.vector.tensor_scalar(
                    ot_sb[:, ot, :], pt2[:],
                    b2_sb[s][:, ot:ot + 1], None,
                    mybir.AluOpType.add)

        def emit_l2(hT, s, tok0, sz):
            # layer 2: outT[o, t] = W2.T @ hT + b2, o-major
            ot_sb = opool.tile([P, OT, sz], F32, tag="ot")
            if interleave:
                # two independent accumulation chains alternating PSUM banks
                pt2s = [ps2.tile([P, sz], F32, tag="ps2", name=f"ps2_{ot}")
                        for ot in range(OT)]
                for kh in range(MH):
                    for ot in range(OT):
                        nc.tensor.matmul(
                            pt2s[ot][:],
                            lhsT=w2_sb[s][:, kh, ot * P:(ot + 1) * P],
                            rhs=hT[:, kh, :sz],
                            start=(kh == 0), stop=(kh == MH - 1))
                for ot in range(OT):
                    emit_bias2(ot_sb, pt2s[ot], s, ot)
            else:
                for ot in range(OT):
                    pt2 = ps2.tile([P, sz], F32, tag="ps2")
                    for kh in range(MH):
                        nc.tensor.matmul(
                            pt2[:],
                            lhsT=w2_sb[s][:, kh, ot * P:(ot + 1) * P],
                            rhs=hT[:, kh, :sz],
                            start=(kh == 0), stop=(kh == MH - 1))
                    emit_bias2(ot_sb, pt2, s, ot)
                    if split_out_dma:
                        # ship each o-half as soon as its bias is done
                        dma_eng = nc.scalar if out_dma_act else nc.sync
                        dma_eng.dma_start(
                            out=out_v[:, ot, tok0:tok0 + sz],
                            in_=ot_sb[:, ot, :])
            # out-DMA optionally on the Activation engine's HWDGE queue so
            # input and output transfers ride two hardware queues
            if not split_out_dma:
                dma_eng = nc.scalar if out_dma_act else nc.sync
                dma_eng.dma_start(out=out_v[:, :, tok0:tok0 + sz],
                                  in_=ot_sb[:])

        for _rep in range(repeat):
            pending = []
            for (s, tok0, sz) in chunk_list:
                xt = xpool.tile([P, KD, sz], F32R, tag="xt")
                nc.sync.dma_start(out=xt[:], in_=xT_v[:, :, tok0:tok0 + sz])

                def emit_relu(hT, pt, s, mh):
                    if mh < relu_acts:
                        nc.scalar.activation(
                            hT[:, mh, :], pt[:], RELU,
                            bias=b1_sb[s][:, mh:mh + 1])
                    else:
                        nc.vector.tensor_scalar(
                            hT[:, mh, :], pt[:],
                            b1_sb[s][:, mh:mh + 1], 0.0,
                            mybir.AluOpType.add, mybir.AluOpType.max)

                hT = hpool.tile([P, MH, sz], F32R, tag="hT")
                if interleave:
                    # pairs of h tiles: two accumulation chains in flight
                    for m in range(MH // 2):
                        pts = [ps1.tile([P, sz], F32, tag="ps1",
                                        name=f"ps1_{j}") for j in range(2)]
                        for kd in range(KD):
                            for j in range(2):
                                nc.tensor.matmul(
                                    pts[j][:],
                                    lhsT=w1_sb[s][:, kd,
                                                  (2 * m + j) * P:
                                                  (2 * m + j + 1) * P],
                                    rhs=xt[:, kd, :],
                                    start=(kd == 0), stop=(kd == KD - 1))
                        for j in range(2):
                            emit_relu(hT, pts[j], s, 2 * m + j)
                else:
                    for mh in range(MH):
                        pt = ps1.tile([P, sz], F32, tag="ps1")
                        for kd in range(KD):
                            nc.tensor.matmul(
                                pt[:],
                                lhsT=w1_sb[s][:, kd, mh * P:(mh + 1) * P],
                                rhs=xt[:, kd, :],
                                start=(kd == 0), stop=(kd == KD - 1))
                        emit_relu(hT, pt, s, mh)

                pending.append((hT, s, tok0, sz))
                if len(pending) > l2_lag:
                    emit_l2(*pending.pop(0))
            for p in pending:
                emit_l2(*p)

    nc.compile()
    return nc


def _route(cat_ids, n_cat):
    counts = np.bincount(cat_ids, minlength=n_cat)
    order = np.argsort(counts, kind="stable")[::-1]
    seg_cats = [order[:N_CORES], order[n_cat - 1:N_CORES - 1:-1]]
    caps = []
    for j in range(2):
        mx = int(counts[seg_cats[j]].max())
        caps.append(max(CHUNK, -(-mx // CHUNK) * CHUNK))
    return seg_cats, caps, counts


_PROG_CACHE = {}


def make_in_maps(x, cat_ids, W1, b1, W2, b2):
    x = np.ascontiguousarray(np.asarray(x, dtype=np.float32))
    cat_ids = np.asarray(cat_ids)
    W1 = np.asarray(W1, dtype=np.float32)
    b1 = np.asarray(b1, dtype=np.float32)
    W2 = np.asarray(W2, dtype=np.float32)
    b2 = np.asarray(b2, dtype=np.float32)

    d_in = x.shape[1]
    n_cat, _, d_h = W1.shape
    d_out = W2.shape[2]

    seg_cats, caps, _counts = _route(cat_ids, n_cat)
    T = sum(caps)

    idx_per_core = []
    in_maps = []
    for i in range(N_CORES):
        cats = [int(seg_cats[0][i]), int(seg_cats[1][i])]
        idxs = [np.flatnonzero(cat_ids == c) for c in cats]
        idx_per_core.append(idxs)
        xT_i = np.zeros((d_in, T), dtype=np.float32)
        off = 0
        for j, (c, idx) in enumerate(zip(cats, idxs)):
            xT_i[:, off:off + len(idx)] = x[idx].T
            off += caps[j]
        # b2T[s][p, ot] = b2[cats[s]][ot*P + p]  (o-major per-partition bias)
        b2T = np.ascontiguousarray(
            b2[cats].reshape(2, d_out // P, P).transpose(0, 2, 1))
        in_maps.append({
            "xT": xT_i,
            "W1": np.ascontiguousarray(W1[cats]),
            "b1": np.ascontiguousarray(b1[cats]),
            "W2": np.ascontiguousarray(W2[cats]),
            "b2T": b2T,
        })
    return in_maps, idx_per_core, caps, (d_in, d_h, d_out)


def unshard_out(results, idx_per_core, caps, B, d_out):
    out_full = np.empty((B, d_out), dtype=np.float32)
    for i in range(N_CORES):
        o = results[i]["outT"]  # [d_out, T]
        off = 0
        for j, idx in enumerate(idx_per_core[i]):
            out_full[idx] = o[:, off:off + len(idx)].T
            off += caps[j]
    return out_full


def kernel(x, cat_ids, W1, b1, W2, b2):
    in_maps, idx_per_core, caps, (d_in, d_h, d_out) = make_in_maps(
        x, cat_ids, W1, b1, W2, b2)

    key = (tuple(caps), d_in, d_h, d_out)
    if key not in _PROG_CACHE:
        _PROG_CACHE[key] = build_program(caps, d_in, d_h, d_out)
    nc = _PROG_CACHE[key]

    res = run_bass_kernel_spmd(nc, in_maps, list(range(N_CORES)))
    return unshard_out(res.results, idx_per_core, caps,
                       np.asarray(x).shape[0], d_out)
